# revision 7
# baseline (speedup 1.0000x reference)
"""BEV pooling (Lift-Splat-Shoot scatter) Trainium2 kernel.

Strategy (8 NeuronCores = 4 batches x 2 y-halves):
  Geometry structure (identity rots/post_rots in this problem): the BEV cell
  of a frustum point depends only on (d, w); the z-keep mask only on (d, h).
  So per batch: h-reduce x[d,:,w,:] over kept h rows -> S1[(d,w), 80], then
  scatter-add ~9.4K columns into the 360x360x80 grid.

  Host (per kernel() call — the NEFF is compiled per invocation, so the whole
  schedule is static):
    - geometry via jax-on-CPU (bit-identical to the reference); masks, cells
    - per shard (batch, y-half): y-major linear cell ids (half 1 y-mirrored
      so both halves share one static schedule; host un-mirrors the output)
    - static window segmentation (DP over 512-cell atoms, window <= 2048
      cells) with per-window tile budgets = max over the 8 shards
    - x_perm gather: [NT, 128, 32*80] f32, zmask-dropped h rows zeroed,
      padded lanes/tiles zero with cell idx -1
  Device (per core, fully static instruction stream):
    - per tile: DMA x-tile; h-reduce -> S1 [128, 80] f32 (DVE tensor_reduce
      or GpSimd add-tree, statically load balanced); hi/lo bf16 split
      (ScalarE cast + sub); one-hot = is_equal(iota16, idx) -> bf16 [128, W]
      (DVE or GpSimd)
    - per window: accumulate its tiles via bf16 matmul pairs into PSUM
      [80, W] f32; ScalarE copy -> SBUF strip; DMA strip -> output region
    - empty regions: DMA from a static zero strip
  Host: assemble output; mirror half-1 rows back.
"""

import numpy as np

# ---------------- problem constants (hardcoded, self-contained) -------------
B, N = 4, 1
IH, IW = 256, 704
FH, FW = 32, 88
C = 80
XB = (-54.0, 54.0, 0.3)
YB = (-54.0, 54.0, 0.3)
ZB = (-10.0, 10.0, 20.0)
DB = (1.0, 60.0, 0.5)
D = int((DB[1] - DB[0]) / DB[2])          # 118
NXG = (360, 360, 1)
HALF = 180 * 360                           # cells per y-half
ATOM = 512
NATOMS = (HALF + ATOM - 1) // ATOM         # 127 (last atom short: 288)
MAXW_ATOMS = 4                             # window <= 2048 cells
SPAN = MAXW_ATOMS * ATOM
HC = FH * C                                # 2560

# engine split fractions (tunable): tile index t goes to gpsimd when
# (t % GPS_MOD) < GPS_K
GPS_MOD, GPS_K = 7, 3                      # ~43% of reduces on gpsimd
OH_GPS_MOD, OH_GPS_K = 10, 0               # one-hots: DVE only


def _geometry(inputs):
    """Frustum -> lidar-frame points, replicated from the reference.
    jax-on-CPU when available (bit-identical to the reference); numpy
    fallback (verified cell-identical on CPU)."""
    args = [np.asarray(inputs[k]) for k in
            ('rots', 'trans', 'intrins', 'post_rots', 'post_trans',
             'lidar2ego_rots', 'lidar2ego_trans', 'extra_rots', 'extra_trans')]
    try:
        import jax
        import jax.numpy as jnp
        cpu = jax.devices("cpu")[0]
        with jax.default_device(cpu):
            ds_ = jnp.broadcast_to(jnp.arange(DB[0], DB[1], DB[2], dtype=jnp.float32)[:, None, None], (D, FH, FW))
            xs = jnp.broadcast_to(jnp.linspace(0.0, IW - 1.0, FW, dtype=jnp.float32)[None, None, :], (D, FH, FW))
            ys = jnp.broadcast_to(jnp.linspace(0.0, IH - 1.0, FH, dtype=jnp.float32)[None, :, None], (D, FH, FW))
            frustum = jnp.stack([xs, ys, ds_], axis=-1)
            rots, trans, intrins, post_rots, post_trans, l2c_rots, l2c_trans, extra_rots, extra_trans = map(jnp.asarray, args)
            pts = frustum[None, None] - post_trans[:, :, None, None, None, :]
            pts = jnp.einsum('bnij,bndhwj->bndhwi', jnp.linalg.inv(post_rots), pts)
            pts = jnp.concatenate([pts[..., :2] * pts[..., 2:3], pts[..., 2:3]], axis=-1)
            combine = jnp.einsum('bnij,bnjk->bnik', rots, jnp.linalg.inv(intrins))
            pts = jnp.einsum('bnij,bndhwj->bndhwi', combine, pts) + trans[:, :, None, None, None, :]
            pts = pts - l2c_trans[:, None, None, None, None, :]
            pts = jnp.einsum('bij,bndhwj->bndhwi', jnp.linalg.inv(l2c_rots), pts)
            pts = jnp.einsum('bij,bndhwj->bndhwi', extra_rots, pts) + extra_trans[:, None, None, None, None, :]
            return np.asarray(pts)
    except Exception:
        pass
    rots, trans, intrins, post_rots, post_trans, l2c_rots, l2c_trans, extra_rots, extra_trans = \
        [a.astype(np.float32) for a in args]
    ds_ = np.broadcast_to(np.arange(DB[0], DB[1], DB[2], dtype=np.float32)[:, None, None], (D, FH, FW))
    xs = np.broadcast_to(np.linspace(0.0, IW - 1.0, FW, dtype=np.float32)[None, None, :], (D, FH, FW))
    ys = np.broadcast_to(np.linspace(0.0, IH - 1.0, FH, dtype=np.float32)[None, :, None], (D, FH, FW))
    frustum = np.stack([xs, ys, ds_], axis=-1)
    pts = frustum[None, None] - post_trans[:, :, None, None, None, :]
    pts = np.einsum('bnij,bndhwj->bndhwi', np.linalg.inv(post_rots), pts)
    pts = np.concatenate([pts[..., :2] * pts[..., 2:3], pts[..., 2:3]], axis=-1)
    combine = np.einsum('bnij,bnjk->bnik', rots, np.linalg.inv(intrins))
    pts = np.einsum('bnij,bndhwj->bndhwi', combine, pts) + trans[:, :, None, None, None, :]
    pts = pts - l2c_trans[:, None, None, None, None, :]
    pts = np.einsum('bij,bndhwj->bndhwi', np.linalg.inv(l2c_rots), pts)
    pts = np.einsum('bij,bndhwj->bndhwi', extra_rots, pts) + extra_trans[:, None, None, None, None, :]
    return pts.astype(np.float32)


def _plan_and_pack(inputs):
    x = np.asarray(inputs['x'])
    geom = _geometry(inputs)                                   # [B,1,D,FH,FW,3]
    DXv = np.array([XB[2], YB[2], ZB[2]], np.float32)
    BXv = np.array([XB[0] + XB[2] / 2, YB[0] + YB[2] / 2, ZB[0] + ZB[2] / 2], np.float32)
    coords = ((geom - (BXv - DXv / 2.0)) / DXv).astype(np.int32)

    cxy = coords[:, 0, :, 0, :, :2]                            # [B, D, FW, 2] (h-indep)
    cz = coords[:, 0, :, :, 0, 2]                              # [B, D, FH]   (w-indep)
    assert (coords[..., 0] == coords[:, :, :, :1, :, 0]).all()
    assert (coords[..., 1] == coords[:, :, :, :1, :, 1]).all()
    assert (coords[..., 2] == coords[:, :, :, :, :1, 2]).all()

    xym = ((cxy[..., 0] >= 0) & (cxy[..., 0] < NXG[0]) &
           (cxy[..., 1] >= 0) & (cxy[..., 1] < NXG[1]))        # [B, D, FW]
    zm = (cz == 0)                                             # [B, D, FH]

    shard_cols = []
    for b in range(B):
        dk, wk = np.nonzero(xym[b])
        cx = cxy[b, dk, wk, 0].astype(np.int64)
        cy = cxy[b, dk, wk, 1].astype(np.int64)
        for half in range(2):
            sel = (cy >= 180 * half) & (cy < 180 * (half + 1))
            cy2 = cy[sel] - 180 * half if half == 0 else 359 - cy[sel]
            lin = cy2 * 360 + cx[sel]
            order = np.argsort(lin, kind='stable')
            shard_cols.append((lin[order], dk[sel][order], wk[sel][order]))

    atom_counts = np.zeros((8, NATOMS), np.int64)
    for s, (lin, _, _) in enumerate(shard_cols):
        w_, c_ = np.unique(lin // ATOM, return_counts=True)
        atom_counts[s, w_] = c_
    pref = np.concatenate([np.zeros((8, 1), np.int64),
                           np.cumsum(atom_counts, axis=1)], axis=1)

    # DP segmentation over the whole half (no slot constraint)
    INF = 1 << 40
    dp = np.full(NATOMS + 1, INF, np.int64)
    dp[0] = 0
    ch = np.zeros(NATOMS + 1, np.int64)
    for i in range(1, NATOMS + 1):
        for w_ in range(1, min(MAXW_ATOMS, i) + 1):
            cols = pref[:, i] - pref[:, i - w_]
            cost = 0 if cols.max() == 0 else int(np.ceil(cols / 128).max())
            if dp[i - w_] + cost < dp[i]:
                dp[i] = dp[i - w_] + cost
                ch[i] = w_
    segs = []
    i = NATOMS
    while i > 0:
        w_ = ch[i]
        segs.append((i - w_, i))
        i -= w_
    segs = segs[::-1]

    # windows: (cell_lo, cell_hi, n_tiles); empties: list of (cell_lo, cell_hi)
    windows = []
    empties = []
    for (sa, sb) in segs:
        clo, chi = sa * ATOM, min(sb * ATOM, HALF)
        cols = pref[:, sb] - pref[:, sa]
        t = 0 if cols.max() == 0 else int(np.ceil(cols / 128).max())
        if t > 0:
            windows.append((clo, chi, t))
        else:
            empties.append((clo, chi))
    # merge adjacent empties, then chunk to <= SPAN
    merged = []
    for (a, bb) in empties:
        if merged and merged[-1][1] == a:
            merged[-1][1] = bb
        else:
            merged.append([a, bb])
    empties = []
    for (a, bb) in merged:
        while a < bb:
            e = min(a + SPAN, bb)
            empties.append((a, e))
            a = e
    NT = sum(t for _, _, t in windows)

    # per-tile max lane count across shards (static DMA/compute partition count)
    nlmax = np.zeros(NT, np.int64)
    for s in range(8):
        lin = shard_cols[s][0]
        ti = 0
        for (clo, chi, t) in windows:
            m0 = np.searchsorted(lin, clo, side='left')
            m1 = np.searchsorted(lin, chi, side='left')
            for k in range(t):
                nl = min(m0 + (k + 1) * 128, m1) - (m0 + k * 128)
                nlmax[ti] = max(nlmax[ti], max(0, nl))
                ti += 1
    nlmax = np.maximum(nlmax, 1)

    x_perm = np.zeros((8, NT, 128, HC), np.float32)
    idxs = np.full((8, 128, NT), -1.0, np.float32)
    xf = x.reshape(B, D, FH, FW, C)
    for s in range(8):
        b = s // 2
        lin, dk, wk = shard_cols[s]
        zmb = zm[b]
        ti = 0
        for (clo, chi, t) in windows:
            m0 = np.searchsorted(lin, clo, side='left')
            m1 = np.searchsorted(lin, chi, side='left')
            for k in range(t):
                lo = m0 + k * 128
                hi = min(m0 + (k + 1) * 128, m1)
                nl = max(0, hi - lo)
                if nl > 0:
                    dsel = dk[lo:hi]
                    wsel = wk[lo:hi]
                    blk = xf[b, dsel, :, wsel, :]              # [nl, FH, C]
                    blk = blk * zmb[dsel][:, :, None]
                    if (ti % GPS_MOD) < GPS_K:                 # gpsimd tree: [h][c]
                        x_perm[s, ti, :nl] = blk.reshape(nl, HC)
                    else:                                      # DVE reduce: [c][h]
                        x_perm[s, ti, :nl] = blk.transpose(0, 2, 1).reshape(nl, HC)
                    idxs[s, :nl, ti] = (lin[lo:hi] - clo).astype(np.float32)
                ti += 1
        assert ti == NT
    iota16 = np.broadcast_to(np.arange(SPAN, dtype=np.int16)[None, :],
                             (128, SPAN)).copy()
    return windows, empties, NT, nlmax, x_perm, idxs, iota16


def _build_program(windows, empties, NT, nlmax):
    import concourse.mybir as mybir
    import concourse.tile as tile
    from concourse import bacc

    F32, BF16, I16 = mybir.dt.float32, mybir.dt.bfloat16, mybir.dt.int16

    nc = bacc.Bacc("TRN2", target_bir_lowering=False, debug=False)
    x_d = nc.dram_tensor("xp", [NT, 128, HC], F32, kind="ExternalInput").ap()
    idx_d = nc.dram_tensor("idx", [128, NT], F32, kind="ExternalInput").ap()
    iota_d = nc.dram_tensor("iota", [128, SPAN], I16, kind="ExternalInput").ap()
    out_d = nc.dram_tensor("out", [C, HALF], F32, kind="ExternalOutput").ap()

    with tile.TileContext(nc) as tc:
        with (
            tc.tile_pool(name="persist", bufs=1) as persist,
            tc.tile_pool(name="xt", bufs=6) as xpool,
            tc.tile_pool(name="oh", bufs=4) as ohpool,
            tc.tile_pool(name="hilo", bufs=4) as hlpool,
            tc.tile_pool(name="strip", bufs=3) as stpool,
            tc.tile_pool(name="psum", bufs=2, space="PSUM") as pspool,
        ):
            iota_t = persist.tile([128, SPAN], I16)
            idx_t = persist.tile([128, NT], F32)
            zero_t = persist.tile([C, SPAN], F32)
            nc.sync.dma_start(iota_t[:], iota_d)
            nc.sync.dma_start(idx_t[:], idx_d)
            nc.gpsimd.memset(zero_t[:], 0.0)

            # empty-region dumps from the static zero strip
            for (clo, chi) in empties:
                nc.scalar.dma_start(out_d[:, clo:chi], zero_t[:, :chi - clo])

            ti = 0
            for (clo, chi, t) in windows:
                W = chi - clo
                ps = pspool.tile([C, SPAN], F32, tag="ps")
                for k in range(t):
                    nl = int(nlmax[ti])
                    xt = xpool.tile([128, HC], F32, tag="xt")
                    nc.sync.dma_start(xt[:nl], x_d[ti, :nl])
                    if (ti % GPS_MOD) < GPS_K:
                        w = HC
                        while w > C:
                            h_ = w // 2
                            nc.gpsimd.tensor_tensor(
                                out=xt[:nl, :h_], in0=xt[:nl, :h_], in1=xt[:nl, h_:w],
                                op=mybir.AluOpType.add)
                            w = h_
                        s1t = xt[:nl, :C]
                    else:
                        s1r = hlpool.tile([128, C], F32, tag="s1r")
                        nc.vector.tensor_reduce(
                            out=s1r[:nl],
                            in_=xt[:nl].rearrange("p (c h) -> p c h", h=FH),
                            axis=mybir.AxisListType.X, op=mybir.AluOpType.add)
                        s1t = s1r[:nl]
                    hi_t = hlpool.tile([128, C], BF16, tag="hi")
                    lo_t = hlpool.tile([128, C], BF16, tag="lo")
                    nc.scalar.activation(out=hi_t[:nl], in_=s1t,
                                         func=mybir.ActivationFunctionType.Copy)
                    nc.gpsimd.tensor_tensor(out=lo_t[:nl], in0=s1t, in1=hi_t[:nl],
                                            op=mybir.AluOpType.subtract)
                    oh = ohpool.tile([128, SPAN], BF16, tag="oh")
                    nc.vector.tensor_scalar(
                        out=oh[:nl, :W], in0=iota_t[:nl, :W],
                        scalar1=idx_t[:nl, ti:ti + 1], scalar2=None,
                        op0=mybir.AluOpType.is_equal)
                    nchunk = (W + 511) // 512
                    for cch in range(nchunk):
                        sl = slice(cch * 512, min((cch + 1) * 512, W))
                        nc.tensor.matmul(out=ps[:, sl], lhsT=hi_t[:nl], rhs=oh[:nl, sl],
                                         start=(k == 0), stop=False)
                        nc.tensor.matmul(out=ps[:, sl], lhsT=lo_t[:nl], rhs=oh[:nl, sl],
                                         start=False, stop=(k == t - 1))
                    ti += 1
                strip = stpool.tile([C, SPAN], F32, tag="strip")
                nc.scalar.activation(out=strip[:, :W], in_=ps[:, :W],
                                     func=mybir.ActivationFunctionType.Copy)
                nc.scalar.dma_start(out_d[:, clo:chi], strip[:, :W])
            assert ti == NT
    nc.compile()
    return nc


def kernel(**inputs) -> np.ndarray:
    import os
    import time
    from concourse.bass_utils import run_bass_kernel_spmd

    windows, empties, NT, nlmax, x_perm, idxs, iota16 = _plan_and_pack(inputs)
    nc = _build_program(windows, empties, NT, nlmax)
    in_maps = [{"xp": x_perm[s], "idx": idxs[s], "iota": iota16} for s in range(8)]
    res = run_bass_kernel_spmd(nc, in_maps, core_ids=list(range(8)))
    out = np.empty((B, C, 360, 360), np.float32)
    for b in range(B):
        lo = res.results[2 * b]["out"].reshape(C, 180, 360)
        hi = res.results[2 * b + 1]["out"].reshape(C, 180, 360)
        out[b, :, :180] = lo
        out[b, :, 180:] = hi[:, ::-1, :]
    return out


# revision 8
# speedup vs baseline: 2.0178x; 2.0178x over previous
"""BEV pooling (Lift-Splat-Shoot scatter) Trainium2 kernel.

Strategy (8 NeuronCores = 4 batches x 2 y-halves):
  Geometry structure (identity rots/post_rots in this problem): the BEV cell
  of a frustum point depends only on (d, w); the z-keep mask only on (d, h).
  So per batch: h-reduce x[d,:,w,:] over kept h rows -> S1[(d,w), 80], then
  scatter-add ~9.4K columns into the 360x360x80 grid.

  Host (per kernel() call — the NEFF is compiled per invocation, so the whole
  schedule is static):
    - geometry via jax-on-CPU (bit-identical to the reference); masks, cells
    - per shard (batch, y-half): y-major linear cell ids (half 1 y-mirrored
      so both halves share one static schedule; host un-mirrors the output)
    - static window segmentation (DP over 512-cell atoms, window <= 2048
      cells) with per-window tile budgets = max over the 8 shards
    - x_perm gather: [NT, 128, 32*80] f32, zmask-dropped h rows zeroed,
      padded lanes/tiles zero with cell idx -1
  Device (per core, fully static instruction stream):
    - per tile: DMA x-tile; h-reduce -> S1 [128, 80] f32 (DVE tensor_reduce
      or GpSimd add-tree, statically load balanced); hi/lo bf16 split
      (ScalarE cast + sub); one-hot = is_equal(iota16, idx) -> bf16 [128, W]
      (DVE or GpSimd)
    - per window: accumulate its tiles via bf16 matmul pairs into PSUM
      [80, W] f32; ScalarE copy -> SBUF strip; DMA strip -> output region
    - empty regions: DMA from a static zero strip
  Host: assemble output; mirror half-1 rows back.
"""

import numpy as np

# ---------------- problem constants (hardcoded, self-contained) -------------
B, N = 4, 1
IH, IW = 256, 704
FH, FW = 32, 88
C = 80
XB = (-54.0, 54.0, 0.3)
YB = (-54.0, 54.0, 0.3)
ZB = (-10.0, 10.0, 20.0)
DB = (1.0, 60.0, 0.5)
D = int((DB[1] - DB[0]) / DB[2])          # 118
NXG = (360, 360, 1)
HALF = 180 * 360                           # cells per y-half
ATOM = 512
NATOMS = (HALF + ATOM - 1) // ATOM         # 127 (last atom short: 288)
MAXW_ATOMS = 4                             # window <= 2048 cells
SPAN = MAXW_ATOMS * ATOM
HC = FH * C                                # 2560

# engine split fractions (tunable): tile index t goes to gpsimd when
# (t % GPS_MOD) < GPS_K
GPS_MOD, GPS_K = 7, 3                      # ~43% of reduces on gpsimd
OH_GPS_MOD, OH_GPS_K = 10, 0               # one-hots: DVE only


def _geometry(inputs):
    """Frustum -> lidar-frame points, replicated from the reference.
    jax-on-CPU when available (bit-identical to the reference); numpy
    fallback (verified cell-identical on CPU)."""
    args = [np.asarray(inputs[k]) for k in
            ('rots', 'trans', 'intrins', 'post_rots', 'post_trans',
             'lidar2ego_rots', 'lidar2ego_trans', 'extra_rots', 'extra_trans')]
    try:
        import jax
        import jax.numpy as jnp
        cpu = jax.devices("cpu")[0]
        with jax.default_device(cpu):
            ds_ = jnp.broadcast_to(jnp.arange(DB[0], DB[1], DB[2], dtype=jnp.float32)[:, None, None], (D, FH, FW))
            xs = jnp.broadcast_to(jnp.linspace(0.0, IW - 1.0, FW, dtype=jnp.float32)[None, None, :], (D, FH, FW))
            ys = jnp.broadcast_to(jnp.linspace(0.0, IH - 1.0, FH, dtype=jnp.float32)[None, :, None], (D, FH, FW))
            frustum = jnp.stack([xs, ys, ds_], axis=-1)
            rots, trans, intrins, post_rots, post_trans, l2c_rots, l2c_trans, extra_rots, extra_trans = map(jnp.asarray, args)
            pts = frustum[None, None] - post_trans[:, :, None, None, None, :]
            pts = jnp.einsum('bnij,bndhwj->bndhwi', jnp.linalg.inv(post_rots), pts)
            pts = jnp.concatenate([pts[..., :2] * pts[..., 2:3], pts[..., 2:3]], axis=-1)
            combine = jnp.einsum('bnij,bnjk->bnik', rots, jnp.linalg.inv(intrins))
            pts = jnp.einsum('bnij,bndhwj->bndhwi', combine, pts) + trans[:, :, None, None, None, :]
            pts = pts - l2c_trans[:, None, None, None, None, :]
            pts = jnp.einsum('bij,bndhwj->bndhwi', jnp.linalg.inv(l2c_rots), pts)
            pts = jnp.einsum('bij,bndhwj->bndhwi', extra_rots, pts) + extra_trans[:, None, None, None, None, :]
            return np.asarray(pts)
    except Exception:
        pass
    rots, trans, intrins, post_rots, post_trans, l2c_rots, l2c_trans, extra_rots, extra_trans = \
        [a.astype(np.float32) for a in args]
    ds_ = np.broadcast_to(np.arange(DB[0], DB[1], DB[2], dtype=np.float32)[:, None, None], (D, FH, FW))
    xs = np.broadcast_to(np.linspace(0.0, IW - 1.0, FW, dtype=np.float32)[None, None, :], (D, FH, FW))
    ys = np.broadcast_to(np.linspace(0.0, IH - 1.0, FH, dtype=np.float32)[None, :, None], (D, FH, FW))
    frustum = np.stack([xs, ys, ds_], axis=-1)
    pts = frustum[None, None] - post_trans[:, :, None, None, None, :]
    pts = np.einsum('bnij,bndhwj->bndhwi', np.linalg.inv(post_rots), pts)
    pts = np.concatenate([pts[..., :2] * pts[..., 2:3], pts[..., 2:3]], axis=-1)
    combine = np.einsum('bnij,bnjk->bnik', rots, np.linalg.inv(intrins))
    pts = np.einsum('bnij,bndhwj->bndhwi', combine, pts) + trans[:, :, None, None, None, :]
    pts = pts - l2c_trans[:, None, None, None, None, :]
    pts = np.einsum('bij,bndhwj->bndhwi', np.linalg.inv(l2c_rots), pts)
    pts = np.einsum('bij,bndhwj->bndhwi', extra_rots, pts) + extra_trans[:, None, None, None, None, :]
    return pts.astype(np.float32)


def _plan_and_pack(inputs):
    x = np.asarray(inputs['x'])
    geom = _geometry(inputs)                                   # [B,1,D,FH,FW,3]
    DXv = np.array([XB[2], YB[2], ZB[2]], np.float32)
    BXv = np.array([XB[0] + XB[2] / 2, YB[0] + YB[2] / 2, ZB[0] + ZB[2] / 2], np.float32)
    coords = ((geom - (BXv - DXv / 2.0)) / DXv).astype(np.int32)

    cxy = coords[:, 0, :, 0, :, :2]                            # [B, D, FW, 2] (h-indep)
    cz = coords[:, 0, :, :, 0, 2]                              # [B, D, FH]   (w-indep)
    assert (coords[..., 0] == coords[:, :, :, :1, :, 0]).all()
    assert (coords[..., 1] == coords[:, :, :, :1, :, 1]).all()
    assert (coords[..., 2] == coords[:, :, :, :, :1, 2]).all()

    xym = ((cxy[..., 0] >= 0) & (cxy[..., 0] < NXG[0]) &
           (cxy[..., 1] >= 0) & (cxy[..., 1] < NXG[1]))        # [B, D, FW]
    zm = (cz == 0)                                             # [B, D, FH]

    shard_cols = []
    for b in range(B):
        dk, wk = np.nonzero(xym[b])
        cx = cxy[b, dk, wk, 0].astype(np.int64)
        cy = cxy[b, dk, wk, 1].astype(np.int64)
        for half in range(2):
            sel = (cy >= 180 * half) & (cy < 180 * (half + 1))
            cy2 = cy[sel] - 180 * half if half == 0 else 359 - cy[sel]
            lin = cy2 * 360 + cx[sel]
            order = np.argsort(lin, kind='stable')
            shard_cols.append((lin[order], dk[sel][order], wk[sel][order]))

    atom_counts = np.zeros((8, NATOMS), np.int64)
    for s, (lin, _, _) in enumerate(shard_cols):
        w_, c_ = np.unique(lin // ATOM, return_counts=True)
        atom_counts[s, w_] = c_
    pref = np.concatenate([np.zeros((8, 1), np.int64),
                           np.cumsum(atom_counts, axis=1)], axis=1)

    # DP segmentation over the whole half (no slot constraint)
    INF = 1 << 40
    dp = np.full(NATOMS + 1, INF, np.int64)
    dp[0] = 0
    ch = np.zeros(NATOMS + 1, np.int64)
    for i in range(1, NATOMS + 1):
        for w_ in range(1, min(MAXW_ATOMS, i) + 1):
            cols = pref[:, i] - pref[:, i - w_]
            cost = 0 if cols.max() == 0 else int(np.ceil(cols / 128).max())
            if dp[i - w_] + cost < dp[i]:
                dp[i] = dp[i - w_] + cost
                ch[i] = w_
    segs = []
    i = NATOMS
    while i > 0:
        w_ = ch[i]
        segs.append((i - w_, i))
        i -= w_
    segs = segs[::-1]

    # windows: (cell_lo, cell_hi, n_tiles); empties: list of (cell_lo, cell_hi)
    windows = []
    empties = []
    for (sa, sb) in segs:
        clo, chi = sa * ATOM, min(sb * ATOM, HALF)
        cols = pref[:, sb] - pref[:, sa]
        t = 0 if cols.max() == 0 else int(np.ceil(cols / 128).max())
        if t > 0:
            windows.append((clo, chi, t))
        else:
            empties.append((clo, chi))
    # merge adjacent empties, then chunk to <= SPAN
    merged = []
    for (a, bb) in empties:
        if merged and merged[-1][1] == a:
            merged[-1][1] = bb
        else:
            merged.append([a, bb])
    empties = []
    for (a, bb) in merged:
        while a < bb:
            e = min(a + SPAN, bb)
            empties.append((a, e))
            a = e
    NT = sum(t for _, _, t in windows)

    # per-tile max lane count across shards (static DMA/compute partition count)
    nlmax = np.zeros(NT, np.int64)
    for s in range(8):
        lin = shard_cols[s][0]
        ti = 0
        for (clo, chi, t) in windows:
            m0 = np.searchsorted(lin, clo, side='left')
            m1 = np.searchsorted(lin, chi, side='left')
            for k in range(t):
                nl = min(m0 + (k + 1) * 128, m1) - (m0 + k * 128)
                nlmax[ti] = max(nlmax[ti], max(0, nl))
                ti += 1
    nlmax = np.maximum(nlmax, 1)

    x_perm = np.zeros((8, NT, 128, HC), np.float32)
    idxs = np.full((8, 128, NT), -1.0, np.float32)
    xf = x.reshape(B, D, FH, FW, C)
    for s in range(8):
        b = s // 2
        lin, dk, wk = shard_cols[s]
        zmb = zm[b]
        ti = 0
        for (clo, chi, t) in windows:
            m0 = np.searchsorted(lin, clo, side='left')
            m1 = np.searchsorted(lin, chi, side='left')
            for k in range(t):
                lo = m0 + k * 128
                hi = min(m0 + (k + 1) * 128, m1)
                nl = max(0, hi - lo)
                if nl > 0:
                    dsel = dk[lo:hi]
                    wsel = wk[lo:hi]
                    blk = xf[b, dsel, :, wsel, :]              # [nl, FH, C]
                    blk = blk * zmb[dsel][:, :, None]
                    if (ti % GPS_MOD) < GPS_K:                 # gpsimd tree: [h][c]
                        x_perm[s, ti, :nl] = blk.reshape(nl, HC)
                    else:                                      # DVE reduce: [c][h]
                        x_perm[s, ti, :nl] = blk.transpose(0, 2, 1).reshape(nl, HC)
                    idxs[s, :nl, ti] = (lin[lo:hi] - clo).astype(np.float32)
                ti += 1
        assert ti == NT
    iota16 = np.broadcast_to(np.arange(SPAN, dtype=np.int16)[None, :],
                             (128, SPAN)).copy()
    return windows, empties, NT, nlmax, x_perm, idxs, iota16


def _build_program(windows, empties, NT, nlmax):
    import concourse.mybir as mybir
    import concourse.tile as tile
    from concourse import bacc

    F32, BF16, I16 = mybir.dt.float32, mybir.dt.bfloat16, mybir.dt.int16

    nc = bacc.Bacc("TRN2", target_bir_lowering=False, debug=False)
    x_d = nc.dram_tensor("xp", [NT, 128, HC], F32, kind="ExternalInput").ap()
    idx_d = nc.dram_tensor("idx", [128, NT], F32, kind="ExternalInput").ap()
    iota_d = nc.dram_tensor("iota", [128, SPAN], I16, kind="ExternalInput").ap()
    out_d = nc.dram_tensor("out", [C, HALF], F32, kind="ExternalOutput").ap()

    with tile.TileContext(nc) as tc:
        with (
            tc.tile_pool(name="persist", bufs=1) as persist,
            tc.tile_pool(name="xt", bufs=6) as xpool,
            tc.tile_pool(name="oh", bufs=4) as ohpool,
            tc.tile_pool(name="hilo", bufs=4) as hlpool,
            tc.tile_pool(name="strip", bufs=3) as stpool,
            tc.tile_pool(name="psum", bufs=2, space="PSUM") as pspool,
        ):
            iota_t = persist.tile([128, SPAN], I16)
            idx_t = persist.tile([128, NT], F32)
            zero_t = persist.tile([C, SPAN], F32)
            nc.sync.dma_start(iota_t[:], iota_d)
            nc.sync.dma_start(idx_t[:], idx_d)
            nc.gpsimd.memset(zero_t[:], 0.0)

            # empty-region dumps from the static zero strip
            for (clo, chi) in empties:
                nc.scalar.dma_start(out_d[:, clo:chi], zero_t[:, :chi - clo])

            ti = 0
            for (clo, chi, t) in windows:
                W = chi - clo
                ps = pspool.tile([C, SPAN], F32, tag="ps")
                for k in range(t):
                    nl = 128
                    xt = xpool.tile([128, HC], F32, tag="xt")
                    nc.sync.dma_start(xt[:], x_d[ti])
                    if (ti % GPS_MOD) < GPS_K:
                        w = HC
                        while w > C:
                            h_ = w // 2
                            nc.gpsimd.tensor_tensor(
                                out=xt[:nl, :h_], in0=xt[:nl, :h_], in1=xt[:nl, h_:w],
                                op=mybir.AluOpType.add)
                            w = h_
                        s1t = xt[:nl, :C]
                    else:
                        s1r = hlpool.tile([128, C], F32, tag="s1r")
                        nc.vector.tensor_reduce(
                            out=s1r[:nl],
                            in_=xt[:nl].rearrange("p (c h) -> p c h", h=FH),
                            axis=mybir.AxisListType.X, op=mybir.AluOpType.add)
                        s1t = s1r[:nl]
                    hi_t = hlpool.tile([128, C], BF16, tag="hi")
                    lo_t = hlpool.tile([128, C], BF16, tag="lo")
                    nc.scalar.activation(out=hi_t[:nl], in_=s1t,
                                         func=mybir.ActivationFunctionType.Copy)
                    nc.gpsimd.tensor_tensor(out=lo_t[:nl], in0=s1t, in1=hi_t[:nl],
                                            op=mybir.AluOpType.subtract)
                    oh = ohpool.tile([128, SPAN], BF16, tag="oh")
                    nc.vector.tensor_scalar(
                        out=oh[:nl, :W], in0=iota_t[:nl, :W],
                        scalar1=idx_t[:nl, ti:ti + 1], scalar2=None,
                        op0=mybir.AluOpType.is_equal)
                    nchunk = (W + 511) // 512
                    for cch in range(nchunk):
                        sl = slice(cch * 512, min((cch + 1) * 512, W))
                        nc.tensor.matmul(out=ps[:, sl], lhsT=hi_t[:nl], rhs=oh[:nl, sl],
                                         start=(k == 0), stop=False)
                        nc.tensor.matmul(out=ps[:, sl], lhsT=lo_t[:nl], rhs=oh[:nl, sl],
                                         start=False, stop=(k == t - 1))
                    ti += 1
                strip = stpool.tile([C, SPAN], F32, tag="strip")
                nc.scalar.activation(out=strip[:, :W], in_=ps[:, :W],
                                     func=mybir.ActivationFunctionType.Copy)
                nc.scalar.dma_start(out_d[:, clo:chi], strip[:, :W])
            assert ti == NT
    nc.compile()
    return nc


def kernel(**inputs) -> np.ndarray:
    import os
    import time
    from concourse.bass_utils import run_bass_kernel_spmd

    windows, empties, NT, nlmax, x_perm, idxs, iota16 = _plan_and_pack(inputs)
    nc = _build_program(windows, empties, NT, nlmax)
    in_maps = [{"xp": x_perm[s], "idx": idxs[s], "iota": iota16} for s in range(8)]
    res = run_bass_kernel_spmd(nc, in_maps, core_ids=list(range(8)))
    out = np.empty((B, C, 360, 360), np.float32)
    for b in range(B):
        lo = res.results[2 * b]["out"].reshape(C, 180, 360)
        hi = res.results[2 * b + 1]["out"].reshape(C, 180, 360)
        out[b, :, :180] = lo
        out[b, :, 180:] = hi[:, ::-1, :]
    return out


# revision 9
# speedup vs baseline: 2.3991x; 1.1890x over previous
"""BEV pooling (Lift-Splat-Shoot scatter) Trainium2 kernel.

Strategy (8 NeuronCores = 4 batches x 2 y-halves):
  Geometry structure (identity rots/post_rots in this problem): the BEV cell
  of a frustum point depends only on (d, w); the z-keep mask only on (d, h).
  So per batch: h-reduce x[d,:,w,:] over kept h rows -> S1[(d,w), 80], then
  scatter-add ~9.4K columns into the 360x360x80 grid.

  Host (per kernel() call — the NEFF is compiled per invocation, so the whole
  schedule is static):
    - geometry via jax-on-CPU (bit-identical to the reference); masks, cells
    - per shard (batch, y-half): y-major linear cell ids (half 1 y-mirrored
      so both halves share one static schedule; host un-mirrors the output)
    - static window segmentation (DP over 512-cell atoms, window <= 2048
      cells) with per-window tile budgets = max over the 8 shards
    - x_perm gather: [NT, 128, 32*80] f32, zmask-dropped h rows zeroed,
      padded lanes/tiles zero with cell idx -1
  Device (per core, fully static instruction stream):
    - per tile: DMA x-tile; h-reduce -> S1 [128, 80] f32 (DVE tensor_reduce
      or GpSimd add-tree, statically load balanced); hi/lo bf16 split
      (ScalarE cast + sub); one-hot = is_equal(iota16, idx) -> bf16 [128, W]
      (DVE or GpSimd)
    - per window: accumulate its tiles via bf16 matmul pairs into PSUM
      [80, W] f32; ScalarE copy -> SBUF strip; DMA strip -> output region
    - empty regions: DMA from a static zero strip
  Host: assemble output; mirror half-1 rows back.
"""

import numpy as np

# ---------------- problem constants (hardcoded, self-contained) -------------
B, N = 4, 1
IH, IW = 256, 704
FH, FW = 32, 88
C = 80
XB = (-54.0, 54.0, 0.3)
YB = (-54.0, 54.0, 0.3)
ZB = (-10.0, 10.0, 20.0)
DB = (1.0, 60.0, 0.5)
D = int((DB[1] - DB[0]) / DB[2])          # 118
NXG = (360, 360, 1)
HALF = 180 * 360                           # cells per y-half
ATOM = 512
NATOMS = (HALF + ATOM - 1) // ATOM         # 127 (last atom short: 288)
MAXW_ATOMS = 4                             # window <= 2048 cells
SPAN = MAXW_ATOMS * ATOM
HC = FH * C                                # 2560

# engine split fractions (tunable): tile index t goes to gpsimd when
# (t % GPS_MOD) < GPS_K
GPS_MOD, GPS_K = 7, 3                      # ~43% of reduces on gpsimd
OH_GPS_MOD, OH_GPS_K = 10, 0               # one-hots: DVE only


def _geometry(inputs):
    """Frustum -> lidar-frame points, replicated from the reference.
    jax-on-CPU when available (bit-identical to the reference); numpy
    fallback (verified cell-identical on CPU)."""
    args = [np.asarray(inputs[k]) for k in
            ('rots', 'trans', 'intrins', 'post_rots', 'post_trans',
             'lidar2ego_rots', 'lidar2ego_trans', 'extra_rots', 'extra_trans')]
    try:
        import jax
        import jax.numpy as jnp
        cpu = jax.devices("cpu")[0]
        with jax.default_device(cpu):
            ds_ = jnp.broadcast_to(jnp.arange(DB[0], DB[1], DB[2], dtype=jnp.float32)[:, None, None], (D, FH, FW))
            xs = jnp.broadcast_to(jnp.linspace(0.0, IW - 1.0, FW, dtype=jnp.float32)[None, None, :], (D, FH, FW))
            ys = jnp.broadcast_to(jnp.linspace(0.0, IH - 1.0, FH, dtype=jnp.float32)[None, :, None], (D, FH, FW))
            frustum = jnp.stack([xs, ys, ds_], axis=-1)
            rots, trans, intrins, post_rots, post_trans, l2c_rots, l2c_trans, extra_rots, extra_trans = map(jnp.asarray, args)
            pts = frustum[None, None] - post_trans[:, :, None, None, None, :]
            pts = jnp.einsum('bnij,bndhwj->bndhwi', jnp.linalg.inv(post_rots), pts)
            pts = jnp.concatenate([pts[..., :2] * pts[..., 2:3], pts[..., 2:3]], axis=-1)
            combine = jnp.einsum('bnij,bnjk->bnik', rots, jnp.linalg.inv(intrins))
            pts = jnp.einsum('bnij,bndhwj->bndhwi', combine, pts) + trans[:, :, None, None, None, :]
            pts = pts - l2c_trans[:, None, None, None, None, :]
            pts = jnp.einsum('bij,bndhwj->bndhwi', jnp.linalg.inv(l2c_rots), pts)
            pts = jnp.einsum('bij,bndhwj->bndhwi', extra_rots, pts) + extra_trans[:, None, None, None, None, :]
            return np.asarray(pts)
    except Exception:
        pass
    rots, trans, intrins, post_rots, post_trans, l2c_rots, l2c_trans, extra_rots, extra_trans = \
        [a.astype(np.float32) for a in args]
    ds_ = np.broadcast_to(np.arange(DB[0], DB[1], DB[2], dtype=np.float32)[:, None, None], (D, FH, FW))
    xs = np.broadcast_to(np.linspace(0.0, IW - 1.0, FW, dtype=np.float32)[None, None, :], (D, FH, FW))
    ys = np.broadcast_to(np.linspace(0.0, IH - 1.0, FH, dtype=np.float32)[None, :, None], (D, FH, FW))
    frustum = np.stack([xs, ys, ds_], axis=-1)
    pts = frustum[None, None] - post_trans[:, :, None, None, None, :]
    pts = np.einsum('bnij,bndhwj->bndhwi', np.linalg.inv(post_rots), pts)
    pts = np.concatenate([pts[..., :2] * pts[..., 2:3], pts[..., 2:3]], axis=-1)
    combine = np.einsum('bnij,bnjk->bnik', rots, np.linalg.inv(intrins))
    pts = np.einsum('bnij,bndhwj->bndhwi', combine, pts) + trans[:, :, None, None, None, :]
    pts = pts - l2c_trans[:, None, None, None, None, :]
    pts = np.einsum('bij,bndhwj->bndhwi', np.linalg.inv(l2c_rots), pts)
    pts = np.einsum('bij,bndhwj->bndhwi', extra_rots, pts) + extra_trans[:, None, None, None, None, :]
    return pts.astype(np.float32)


def _plan_and_pack(inputs):
    x = np.asarray(inputs['x'])
    geom = _geometry(inputs)                                   # [B,1,D,FH,FW,3]
    DXv = np.array([XB[2], YB[2], ZB[2]], np.float32)
    BXv = np.array([XB[0] + XB[2] / 2, YB[0] + YB[2] / 2, ZB[0] + ZB[2] / 2], np.float32)
    coords = ((geom - (BXv - DXv / 2.0)) / DXv).astype(np.int32)

    cxy = coords[:, 0, :, 0, :, :2]                            # [B, D, FW, 2] (h-indep)
    cz = coords[:, 0, :, :, 0, 2]                              # [B, D, FH]   (w-indep)
    assert (coords[..., 0] == coords[:, :, :, :1, :, 0]).all()
    assert (coords[..., 1] == coords[:, :, :, :1, :, 1]).all()
    assert (coords[..., 2] == coords[:, :, :, :, :1, 2]).all()

    xym = ((cxy[..., 0] >= 0) & (cxy[..., 0] < NXG[0]) &
           (cxy[..., 1] >= 0) & (cxy[..., 1] < NXG[1]))        # [B, D, FW]
    zm = (cz == 0)                                             # [B, D, FH]

    shard_cols = []
    for b in range(B):
        dk, wk = np.nonzero(xym[b])
        cx = cxy[b, dk, wk, 0].astype(np.int64)
        cy = cxy[b, dk, wk, 1].astype(np.int64)
        for half in range(2):
            sel = (cy >= 180 * half) & (cy < 180 * (half + 1))
            cy2 = cy[sel] - 180 * half if half == 0 else 359 - cy[sel]
            lin = cy2 * 360 + cx[sel]
            order = np.argsort(lin, kind='stable')
            shard_cols.append((lin[order], dk[sel][order], wk[sel][order]))

    atom_counts = np.zeros((8, NATOMS), np.int64)
    for s, (lin, _, _) in enumerate(shard_cols):
        w_, c_ = np.unique(lin // ATOM, return_counts=True)
        atom_counts[s, w_] = c_
    pref = np.concatenate([np.zeros((8, 1), np.int64),
                           np.cumsum(atom_counts, axis=1)], axis=1)

    # DP segmentation over the whole half (no slot constraint)
    INF = 1 << 40
    dp = np.full(NATOMS + 1, INF, np.int64)
    dp[0] = 0
    ch = np.zeros(NATOMS + 1, np.int64)
    for i in range(1, NATOMS + 1):
        for w_ in range(1, min(MAXW_ATOMS, i) + 1):
            cols = pref[:, i] - pref[:, i - w_]
            cost = 0 if cols.max() == 0 else int(np.ceil(cols / 128).max())
            if dp[i - w_] + cost < dp[i]:
                dp[i] = dp[i - w_] + cost
                ch[i] = w_
    segs = []
    i = NATOMS
    while i > 0:
        w_ = ch[i]
        segs.append((i - w_, i))
        i -= w_
    segs = segs[::-1]

    # windows: (cell_lo, cell_hi, n_tiles); empties: list of (cell_lo, cell_hi)
    windows = []
    empties = []
    for (sa, sb) in segs:
        clo, chi = sa * ATOM, min(sb * ATOM, HALF)
        cols = pref[:, sb] - pref[:, sa]
        t = 0 if cols.max() == 0 else int(np.ceil(cols / 128).max())
        if t > 0:
            windows.append((clo, chi, t))
        else:
            empties.append((clo, chi))
    # merge adjacent empties, then chunk to <= SPAN
    merged = []
    for (a, bb) in empties:
        if merged and merged[-1][1] == a:
            merged[-1][1] = bb
        else:
            merged.append([a, bb])
    empties = []
    for (a, bb) in merged:
        while a < bb:
            e = min(a + SPAN, bb)
            empties.append((a, e))
            a = e
    NT = sum(t for _, _, t in windows)

    # per-tile max lane count across shards (static DMA/compute partition count)
    nlmax = np.zeros(NT, np.int64)
    for s in range(8):
        lin = shard_cols[s][0]
        ti = 0
        for (clo, chi, t) in windows:
            m0 = np.searchsorted(lin, clo, side='left')
            m1 = np.searchsorted(lin, chi, side='left')
            for k in range(t):
                nl = min(m0 + (k + 1) * 128, m1) - (m0 + k * 128)
                nlmax[ti] = max(nlmax[ti], max(0, nl))
                ti += 1
    nlmax = np.maximum(nlmax, 1)

    x_perm = np.zeros((8, NT, 128, HC), np.float32)
    idxs = np.full((8, 128, NT), -1.0, np.float32)
    xf = x.reshape(B, D, FH, FW, C)
    for s in range(8):
        b = s // 2
        lin, dk, wk = shard_cols[s]
        zmb = zm[b]
        ti = 0
        for (clo, chi, t) in windows:
            m0 = np.searchsorted(lin, clo, side='left')
            m1 = np.searchsorted(lin, chi, side='left')
            for k in range(t):
                lo = m0 + k * 128
                hi = min(m0 + (k + 1) * 128, m1)
                nl = max(0, hi - lo)
                if nl > 0:
                    dsel = dk[lo:hi]
                    wsel = wk[lo:hi]
                    blk = xf[b, dsel, :, wsel, :]              # [nl, FH, C]
                    blk = blk * zmb[dsel][:, :, None]
                    if (ti % GPS_MOD) < GPS_K:                 # gpsimd tree: [h][c]
                        x_perm[s, ti, :nl] = blk.reshape(nl, HC)
                    else:                                      # DVE reduce: [c][h]
                        x_perm[s, ti, :nl] = blk.transpose(0, 2, 1).reshape(nl, HC)
                    idxs[s, :nl, ti] = (lin[lo:hi] - clo).astype(np.float32)
                ti += 1
        assert ti == NT
    iota16 = np.broadcast_to(np.arange(SPAN, dtype=np.int16)[None, :],
                             (128, SPAN)).copy()
    return windows, empties, NT, nlmax, x_perm, idxs, iota16


def _build_program(windows, empties, NT, nlmax):
    import concourse.mybir as mybir
    import concourse.tile as tile
    from concourse import bacc

    F32, BF16, I16 = mybir.dt.float32, mybir.dt.bfloat16, mybir.dt.int16

    nc = bacc.Bacc("TRN2", target_bir_lowering=False, debug=False)
    x_d = nc.dram_tensor("xp", [NT, 128, HC], F32, kind="ExternalInput").ap()
    idx_d = nc.dram_tensor("idx", [128, NT], F32, kind="ExternalInput").ap()
    iota_d = nc.dram_tensor("iota", [128, SPAN], I16, kind="ExternalInput").ap()
    out_d = nc.dram_tensor("out", [C, HALF], F32, kind="ExternalOutput").ap()

    with tile.TileContext(nc) as tc:
        with (
            tc.tile_pool(name="persist", bufs=1) as persist,
            tc.tile_pool(name="xt", bufs=6) as xpool,
            tc.tile_pool(name="oh", bufs=4) as ohpool,
            tc.tile_pool(name="hilo", bufs=4) as hlpool,
            tc.tile_pool(name="strip", bufs=3) as stpool,
            tc.tile_pool(name="psum", bufs=2, space="PSUM") as pspool,
        ):
            iota_t = persist.tile([128, SPAN], I16)
            idx_t = persist.tile([128, NT], F32)
            zero_t = persist.tile([C, SPAN], F32)
            nc.sync.dma_start(iota_t[:], iota_d)
            nc.sync.dma_start(idx_t[:], idx_d)
            nc.gpsimd.memset(zero_t[:], 0.0)

            # empty-region dumps from the static zero strip
            for (clo, chi) in empties:
                nc.scalar.dma_start(out_d[:, clo:chi], zero_t[:, :chi - clo])

            ti = 0
            for (clo, chi, t) in windows:
                W = chi - clo
                ps = pspool.tile([C, SPAN], F32, tag="ps")
                for k in range(t):
                    xt = xpool.tile([128, HC], F32, tag="xt")
                    nc.sync.dma_start(xt[:], x_d[ti])
                    if (ti % GPS_MOD) < GPS_K:
                        w = HC
                        while w > C:
                            h_ = w // 2
                            nc.gpsimd.tensor_tensor(
                                out=xt[:, :h_], in0=xt[:, :h_], in1=xt[:, h_:w],
                                op=mybir.AluOpType.add)
                            w = h_
                        s1t = xt[:, :C]
                    else:
                        s1r = hlpool.tile([128, C], F32, tag="s1r")
                        nc.vector.tensor_reduce(
                            out=s1r[:],
                            in_=xt[:].rearrange("p (c h) -> p c h", h=FH),
                            axis=mybir.AxisListType.X, op=mybir.AluOpType.add)
                        s1t = s1r[:]
                    hi_t = hlpool.tile([128, C], BF16, tag="hi")
                    lo_t = hlpool.tile([128, C], BF16, tag="lo")
                    nc.scalar.activation(out=hi_t[:], in_=s1t,
                                         func=mybir.ActivationFunctionType.Copy)
                    nc.gpsimd.tensor_tensor(out=lo_t[:], in0=s1t, in1=hi_t[:],
                                            op=mybir.AluOpType.subtract)
                    oh = ohpool.tile([128, SPAN], BF16, tag="oh")
                    nc.vector.tensor_scalar(
                        out=oh[:, :W], in0=iota_t[:, :W],
                        scalar1=idx_t[:, ti:ti + 1], scalar2=None,
                        op0=mybir.AluOpType.is_equal)
                    nchunk = (W + 511) // 512
                    for cch in range(nchunk):
                        sl = slice(cch * 512, min((cch + 1) * 512, W))
                        nc.tensor.matmul(out=ps[:, sl], lhsT=hi_t[:], rhs=oh[:, sl],
                                         start=(k == 0), stop=False)
                        nc.tensor.matmul(out=ps[:, sl], lhsT=lo_t[:], rhs=oh[:, sl],
                                         start=False, stop=(k == t - 1))
                    ti += 1
                strip = stpool.tile([C, SPAN], F32, tag="strip")
                nc.scalar.activation(out=strip[:, :W], in_=ps[:, :W],
                                     func=mybir.ActivationFunctionType.Copy)
                nc.scalar.dma_start(out_d[:, clo:chi], strip[:, :W])
            assert ti == NT
    nc.compile()
    return nc


def kernel(**inputs) -> np.ndarray:
    import os
    import time
    from concourse.bass_utils import run_bass_kernel_spmd

    windows, empties, NT, nlmax, x_perm, idxs, iota16 = _plan_and_pack(inputs)
    nc = _build_program(windows, empties, NT, nlmax)
    in_maps = [{"xp": x_perm[s], "idx": idxs[s], "iota": iota16} for s in range(8)]
    res = run_bass_kernel_spmd(nc, in_maps, core_ids=list(range(8)))
    out = np.empty((B, C, 360, 360), np.float32)
    for b in range(B):
        lo = res.results[2 * b]["out"].reshape(C, 180, 360)
        hi = res.results[2 * b + 1]["out"].reshape(C, 180, 360)
        out[b, :, :180] = lo
        out[b, :, 180:] = hi[:, ::-1, :]
    return out


# revision 10
# speedup vs baseline: 2.4975x; 1.0410x over previous
"""BEV pooling (Lift-Splat-Shoot scatter) Trainium2 kernel.

Strategy (8 NeuronCores = 4 batches x 2 y-halves):
  Geometry structure (identity rots/post_rots in this problem): the BEV cell
  of a frustum point depends only on (d, w); the z-keep mask only on (d, h).
  So per batch: h-reduce x[d,:,w,:] over kept h rows -> S1[(d,w), 80], then
  scatter-add ~9.4K columns into the 360x360x80 grid.

  Host (per kernel() call — the NEFF is compiled per invocation, so the whole
  schedule is static):
    - geometry via jax-on-CPU (bit-identical to the reference); masks, cells
    - per shard (batch, y-half): y-major linear cell ids (half 1 y-mirrored
      so both halves share one static schedule; host un-mirrors the output)
    - static window segmentation (DP over 512-cell atoms, window <= 2048
      cells) with per-window tile budgets = max over the 8 shards
    - x_perm gather: [NT, 128, 32*80] f32, zmask-dropped h rows zeroed,
      padded lanes/tiles zero with cell idx -1
  Device (per core, fully static instruction stream):
    - per tile: DMA x-tile; h-reduce -> S1 [128, 80] f32 (DVE tensor_reduce
      or GpSimd add-tree, statically load balanced); hi/lo bf16 split
      (ScalarE cast + sub); one-hot = is_equal(iota16, idx) -> bf16 [128, W]
      (DVE or GpSimd)
    - per window: accumulate its tiles via bf16 matmul pairs into PSUM
      [80, W] f32; ScalarE copy -> SBUF strip; DMA strip -> output region
    - empty regions: DMA from a static zero strip
  Host: assemble output; mirror half-1 rows back.
"""

import numpy as np

# ---------------- problem constants (hardcoded, self-contained) -------------
B, N = 4, 1
IH, IW = 256, 704
FH, FW = 32, 88
C = 80
XB = (-54.0, 54.0, 0.3)
YB = (-54.0, 54.0, 0.3)
ZB = (-10.0, 10.0, 20.0)
DB = (1.0, 60.0, 0.5)
D = int((DB[1] - DB[0]) / DB[2])          # 118
NXG = (360, 360, 1)
HALF = 180 * 360                           # cells per y-half
ATOM = 512
NATOMS = (HALF + ATOM - 1) // ATOM         # 127 (last atom short: 288)
MAXW_ATOMS = 4                             # window <= 2048 cells
SPAN = MAXW_ATOMS * ATOM
HC = FH * C                                # 2560

# engine split fractions (tunable): tile index t goes to gpsimd when
# (t % GPS_MOD) < GPS_K
GPS_MOD, GPS_K = 17, 6                     # ~35% of reduces on gpsimd
OH_GPS_MOD, OH_GPS_K = 10, 0               # one-hots: DVE only


def _geometry(inputs):
    """Frustum -> lidar-frame points, replicated from the reference.
    jax-on-CPU when available (bit-identical to the reference); numpy
    fallback (verified cell-identical on CPU)."""
    args = [np.asarray(inputs[k]) for k in
            ('rots', 'trans', 'intrins', 'post_rots', 'post_trans',
             'lidar2ego_rots', 'lidar2ego_trans', 'extra_rots', 'extra_trans')]
    try:
        import jax
        import jax.numpy as jnp
        cpu = jax.devices("cpu")[0]
        with jax.default_device(cpu):
            ds_ = jnp.broadcast_to(jnp.arange(DB[0], DB[1], DB[2], dtype=jnp.float32)[:, None, None], (D, FH, FW))
            xs = jnp.broadcast_to(jnp.linspace(0.0, IW - 1.0, FW, dtype=jnp.float32)[None, None, :], (D, FH, FW))
            ys = jnp.broadcast_to(jnp.linspace(0.0, IH - 1.0, FH, dtype=jnp.float32)[None, :, None], (D, FH, FW))
            frustum = jnp.stack([xs, ys, ds_], axis=-1)
            rots, trans, intrins, post_rots, post_trans, l2c_rots, l2c_trans, extra_rots, extra_trans = map(jnp.asarray, args)
            pts = frustum[None, None] - post_trans[:, :, None, None, None, :]
            pts = jnp.einsum('bnij,bndhwj->bndhwi', jnp.linalg.inv(post_rots), pts)
            pts = jnp.concatenate([pts[..., :2] * pts[..., 2:3], pts[..., 2:3]], axis=-1)
            combine = jnp.einsum('bnij,bnjk->bnik', rots, jnp.linalg.inv(intrins))
            pts = jnp.einsum('bnij,bndhwj->bndhwi', combine, pts) + trans[:, :, None, None, None, :]
            pts = pts - l2c_trans[:, None, None, None, None, :]
            pts = jnp.einsum('bij,bndhwj->bndhwi', jnp.linalg.inv(l2c_rots), pts)
            pts = jnp.einsum('bij,bndhwj->bndhwi', extra_rots, pts) + extra_trans[:, None, None, None, None, :]
            return np.asarray(pts)
    except Exception:
        pass
    rots, trans, intrins, post_rots, post_trans, l2c_rots, l2c_trans, extra_rots, extra_trans = \
        [a.astype(np.float32) for a in args]
    ds_ = np.broadcast_to(np.arange(DB[0], DB[1], DB[2], dtype=np.float32)[:, None, None], (D, FH, FW))
    xs = np.broadcast_to(np.linspace(0.0, IW - 1.0, FW, dtype=np.float32)[None, None, :], (D, FH, FW))
    ys = np.broadcast_to(np.linspace(0.0, IH - 1.0, FH, dtype=np.float32)[None, :, None], (D, FH, FW))
    frustum = np.stack([xs, ys, ds_], axis=-1)
    pts = frustum[None, None] - post_trans[:, :, None, None, None, :]
    pts = np.einsum('bnij,bndhwj->bndhwi', np.linalg.inv(post_rots), pts)
    pts = np.concatenate([pts[..., :2] * pts[..., 2:3], pts[..., 2:3]], axis=-1)
    combine = np.einsum('bnij,bnjk->bnik', rots, np.linalg.inv(intrins))
    pts = np.einsum('bnij,bndhwj->bndhwi', combine, pts) + trans[:, :, None, None, None, :]
    pts = pts - l2c_trans[:, None, None, None, None, :]
    pts = np.einsum('bij,bndhwj->bndhwi', np.linalg.inv(l2c_rots), pts)
    pts = np.einsum('bij,bndhwj->bndhwi', extra_rots, pts) + extra_trans[:, None, None, None, None, :]
    return pts.astype(np.float32)


def _plan_and_pack(inputs):
    x = np.asarray(inputs['x'])
    geom = _geometry(inputs)                                   # [B,1,D,FH,FW,3]
    DXv = np.array([XB[2], YB[2], ZB[2]], np.float32)
    BXv = np.array([XB[0] + XB[2] / 2, YB[0] + YB[2] / 2, ZB[0] + ZB[2] / 2], np.float32)
    coords = ((geom - (BXv - DXv / 2.0)) / DXv).astype(np.int32)

    cxy = coords[:, 0, :, 0, :, :2]                            # [B, D, FW, 2] (h-indep)
    cz = coords[:, 0, :, :, 0, 2]                              # [B, D, FH]   (w-indep)
    assert (coords[..., 0] == coords[:, :, :, :1, :, 0]).all()
    assert (coords[..., 1] == coords[:, :, :, :1, :, 1]).all()
    assert (coords[..., 2] == coords[:, :, :, :, :1, 2]).all()

    xym = ((cxy[..., 0] >= 0) & (cxy[..., 0] < NXG[0]) &
           (cxy[..., 1] >= 0) & (cxy[..., 1] < NXG[1]))        # [B, D, FW]
    zm = (cz == 0)                                             # [B, D, FH]

    shard_cols = []
    for b in range(B):
        dk, wk = np.nonzero(xym[b])
        cx = cxy[b, dk, wk, 0].astype(np.int64)
        cy = cxy[b, dk, wk, 1].astype(np.int64)
        for half in range(2):
            sel = (cy >= 180 * half) & (cy < 180 * (half + 1))
            cy2 = cy[sel] - 180 * half if half == 0 else 359 - cy[sel]
            lin = cy2 * 360 + cx[sel]
            order = np.argsort(lin, kind='stable')
            shard_cols.append((lin[order], dk[sel][order], wk[sel][order]))

    atom_counts = np.zeros((8, NATOMS), np.int64)
    for s, (lin, _, _) in enumerate(shard_cols):
        w_, c_ = np.unique(lin // ATOM, return_counts=True)
        atom_counts[s, w_] = c_
    pref = np.concatenate([np.zeros((8, 1), np.int64),
                           np.cumsum(atom_counts, axis=1)], axis=1)

    # DP segmentation over the whole half (no slot constraint)
    INF = 1 << 40
    dp = np.full(NATOMS + 1, INF, np.int64)
    dp[0] = 0
    ch = np.zeros(NATOMS + 1, np.int64)
    for i in range(1, NATOMS + 1):
        for w_ in range(1, min(MAXW_ATOMS, i) + 1):
            cols = pref[:, i] - pref[:, i - w_]
            cost = 0 if cols.max() == 0 else int(np.ceil(cols / 128).max())
            if dp[i - w_] + cost < dp[i]:
                dp[i] = dp[i - w_] + cost
                ch[i] = w_
    segs = []
    i = NATOMS
    while i > 0:
        w_ = ch[i]
        segs.append((i - w_, i))
        i -= w_
    segs = segs[::-1]

    # windows: (cell_lo, cell_hi, n_tiles); empties: list of (cell_lo, cell_hi)
    windows = []
    empties = []
    for (sa, sb) in segs:
        clo, chi = sa * ATOM, min(sb * ATOM, HALF)
        cols = pref[:, sb] - pref[:, sa]
        t = 0 if cols.max() == 0 else int(np.ceil(cols / 128).max())
        if t > 0:
            windows.append((clo, chi, t))
        else:
            empties.append((clo, chi))
    # merge adjacent empties, then chunk to <= SPAN
    merged = []
    for (a, bb) in empties:
        if merged and merged[-1][1] == a:
            merged[-1][1] = bb
        else:
            merged.append([a, bb])
    empties = []
    for (a, bb) in merged:
        while a < bb:
            e = min(a + SPAN, bb)
            empties.append((a, e))
            a = e
    # heavy windows first: the read stream then ends with cheap windows and
    # the DMA tail isn't stuck waiting for a deep compute pipeline.
    windows.sort(key=lambda w: -w[2])
    NT = sum(t for _, _, t in windows)

    # per-tile max lane count across shards (static DMA/compute partition count)
    nlmax = np.zeros(NT, np.int64)
    for s in range(8):
        lin = shard_cols[s][0]
        ti = 0
        for (clo, chi, t) in windows:
            m0 = np.searchsorted(lin, clo, side='left')
            m1 = np.searchsorted(lin, chi, side='left')
            for k in range(t):
                nl = min(m0 + (k + 1) * 128, m1) - (m0 + k * 128)
                nlmax[ti] = max(nlmax[ti], max(0, nl))
                ti += 1
    nlmax = np.maximum(nlmax, 1)

    x_perm = np.zeros((8, NT, 128, HC), np.float32)
    idxs = np.full((8, 128, NT), -1.0, np.float32)
    xf = x.reshape(B, D, FH, FW, C)
    for s in range(8):
        b = s // 2
        lin, dk, wk = shard_cols[s]
        zmb = zm[b]
        ti = 0
        for (clo, chi, t) in windows:
            m0 = np.searchsorted(lin, clo, side='left')
            m1 = np.searchsorted(lin, chi, side='left')
            for k in range(t):
                lo = m0 + k * 128
                hi = min(m0 + (k + 1) * 128, m1)
                nl = max(0, hi - lo)
                if nl > 0:
                    dsel = dk[lo:hi]
                    wsel = wk[lo:hi]
                    blk = xf[b, dsel, :, wsel, :]              # [nl, FH, C]
                    blk = blk * zmb[dsel][:, :, None]
                    if (ti % GPS_MOD) < GPS_K:                 # gpsimd tree: [h][c]
                        x_perm[s, ti, :nl] = blk.reshape(nl, HC)
                    else:                                      # DVE reduce: [c][h]
                        x_perm[s, ti, :nl] = blk.transpose(0, 2, 1).reshape(nl, HC)
                    idxs[s, :nl, ti] = (lin[lo:hi] - clo).astype(np.float32)
                ti += 1
        assert ti == NT
    iota16 = np.broadcast_to(np.arange(SPAN, dtype=np.int16)[None, :],
                             (128, SPAN)).copy()
    return windows, empties, NT, nlmax, x_perm, idxs, iota16


def _build_program(windows, empties, NT, nlmax):
    import concourse.mybir as mybir
    import concourse.tile as tile
    from concourse import bacc

    F32, BF16, I16 = mybir.dt.float32, mybir.dt.bfloat16, mybir.dt.int16

    nc = bacc.Bacc("TRN2", target_bir_lowering=False, debug=False)
    x_d = nc.dram_tensor("xp", [NT, 128, HC], F32, kind="ExternalInput").ap()
    idx_d = nc.dram_tensor("idx", [128, NT], F32, kind="ExternalInput").ap()
    iota_d = nc.dram_tensor("iota", [128, SPAN], I16, kind="ExternalInput").ap()
    out_d = nc.dram_tensor("out", [C, HALF], F32, kind="ExternalOutput").ap()

    with tile.TileContext(nc) as tc:
        with (
            tc.tile_pool(name="persist", bufs=1) as persist,
            tc.tile_pool(name="xt", bufs=6) as xpool,
            tc.tile_pool(name="oh", bufs=4) as ohpool,
            tc.tile_pool(name="hilo", bufs=4) as hlpool,
            tc.tile_pool(name="strip", bufs=3) as stpool,
            tc.tile_pool(name="psum", bufs=2, space="PSUM") as pspool,
        ):
            iota_t = persist.tile([128, SPAN], I16)
            idx_t = persist.tile([128, NT], F32)
            zero_t = persist.tile([C, SPAN], F32)
            nc.sync.dma_start(iota_t[:], iota_d)
            nc.sync.dma_start(idx_t[:], idx_d)
            nc.gpsimd.memset(zero_t[:], 0.0)

            # empty-region dumps from the static zero strip
            for (clo, chi) in empties:
                nc.scalar.dma_start(out_d[:, clo:chi], zero_t[:, :chi - clo])

            ti = 0
            for (clo, chi, t) in windows:
                W = chi - clo
                ps = pspool.tile([C, SPAN], F32, tag="ps")
                for k in range(t):
                    xt = xpool.tile([128, HC], F32, tag="xt")
                    nc.sync.dma_start(xt[:], x_d[ti])
                    if (ti % GPS_MOD) < GPS_K:
                        w = HC
                        while w > C:
                            h_ = w // 2
                            nc.gpsimd.tensor_tensor(
                                out=xt[:, :h_], in0=xt[:, :h_], in1=xt[:, h_:w],
                                op=mybir.AluOpType.add)
                            w = h_
                        s1t = xt[:, :C]
                    else:
                        s1r = hlpool.tile([128, C], F32, tag="s1r")
                        nc.vector.tensor_reduce(
                            out=s1r[:],
                            in_=xt[:].rearrange("p (c h) -> p c h", h=FH),
                            axis=mybir.AxisListType.X, op=mybir.AluOpType.add)
                        s1t = s1r[:]
                    hi_t = hlpool.tile([128, C], BF16, tag="hi")
                    lo_t = hlpool.tile([128, C], BF16, tag="lo")
                    nc.scalar.activation(out=hi_t[:], in_=s1t,
                                         func=mybir.ActivationFunctionType.Copy)
                    nc.gpsimd.tensor_tensor(out=lo_t[:], in0=s1t, in1=hi_t[:],
                                            op=mybir.AluOpType.subtract)
                    oh = ohpool.tile([128, SPAN], BF16, tag="oh")
                    nc.vector.tensor_scalar(
                        out=oh[:, :W], in0=iota_t[:, :W],
                        scalar1=idx_t[:, ti:ti + 1], scalar2=None,
                        op0=mybir.AluOpType.is_equal)
                    nchunk = (W + 511) // 512
                    for cch in range(nchunk):
                        sl = slice(cch * 512, min((cch + 1) * 512, W))
                        nc.tensor.matmul(out=ps[:, sl], lhsT=hi_t[:], rhs=oh[:, sl],
                                         start=(k == 0), stop=False)
                        nc.tensor.matmul(out=ps[:, sl], lhsT=lo_t[:], rhs=oh[:, sl],
                                         start=False, stop=(k == t - 1))
                    ti += 1
                strip = stpool.tile([C, SPAN], F32, tag="strip")
                nc.scalar.activation(out=strip[:, :W], in_=ps[:, :W],
                                     func=mybir.ActivationFunctionType.Copy)
                nc.scalar.dma_start(out_d[:, clo:chi], strip[:, :W])
            assert ti == NT
    nc.compile()
    return nc


def kernel(**inputs) -> np.ndarray:
    import os
    import time
    from concourse.bass_utils import run_bass_kernel_spmd

    windows, empties, NT, nlmax, x_perm, idxs, iota16 = _plan_and_pack(inputs)
    nc = _build_program(windows, empties, NT, nlmax)
    in_maps = [{"xp": x_perm[s], "idx": idxs[s], "iota": iota16} for s in range(8)]
    res = run_bass_kernel_spmd(nc, in_maps, core_ids=list(range(8)))
    out = np.empty((B, C, 360, 360), np.float32)
    for b in range(B):
        lo = res.results[2 * b]["out"].reshape(C, 180, 360)
        hi = res.results[2 * b + 1]["out"].reshape(C, 180, 360)
        out[b, :, :180] = lo
        out[b, :, 180:] = hi[:, ::-1, :]
    return out
